# revision 43
# baseline (speedup 1.0000x reference)
"""Trainium2 Bass kernel for nn_Aggregation (SAN-style local aggregation).

out[n, g*32+cc, h, w] = sum_{kh,kw} input[n, g*32+cc, h-3+kh, w-3+kw] * weight[n, cc, kh*7+kw, h, w]

Sharding: data-parallel over batch N=16 across 8 NeuronCores (2 images/core).

v3 layout (per core):
  partition p = cc*4 + blk   (cc in [0,32): weight channel, blk in [0,4): block of 8 output rows)
  in_nc[p][n, g, rho, w]: rho = 1 + r, r in [0,14) the padded window rows
    (lead row rho=0 and spare row rho=15 absorb column spill), w unpadded.
  Input is HOST-prepared: fp16, halo rows pre-zeroed, exact SBUF layout ->
  one plain 2D HWDGE DMA per image (no cast, no memset, no SWDGE).
  Weight is host-permuted to [n, (cc blk), kw, kh, hb, w] fp16 with the edge
  output-columns of each kw-tap zeroed host-side (exact: those weights
  multiply conv padding zeros in the reference).
  Engines: DVE computes all 14 fat tensor_tensor multiplies (free dims
  kh:7, g:8, (hb w):256 -- 2x_1P mode, fully dense back-to-back). All DMAs
  are HWDGE (sync/scalar rings) so nothing contends with the DVE; the 12
  non-critical weight chunks are dependency-gated behind the ident load so
  the n=0 input + first chunk get the full SDMA bandwidth at startup.
  Products are fp16; the Tensor engine accumulates all taps into an fp32
  PSUM accumulator via identity matmuls, chased by DVE/ACT PSUM evictions
  and per-quarter output stores.

  Measured (8 cores, this config): ~128.9-129.6us HW exec (cold device;
  +-1us run-to-run DMA-receipt jitter). Breakdown: ~7.2us fixed engine
  preamble, ~6.1us first-data DMA latency (latency-bound: halving the
  gating bytes moves the gate <0.1us), 106.4us DVE multiplies (the
  2 elem/cycle/partition 2x_1P roofline -- 25.7M products/core), ~1.1us
  PE drain (single-kh final units), ~8us evict/store/receipt/cleanup. The chip thermally
  throttles ~0.83x (both PE and DVE; ~153us) after many back-to-back
  runs; ~5min idle restores full clocks. Things measured WORSE: gpsimd
  tensor_tensor offload (shared-SBUF-port contention slows DVE ~0.6x per
  gpsimd-busy-us), SWDGE/gpsimd DMA issue (late descriptor gen + big
  drains), splitting the n=0 input DMA across both HWDGE rings.
"""

import numpy as np

N, C, H, W = 16, 256, 32, 32
K, PAD = 7, 3
CC, G = 32, 8
KK = K * K
NCORES = 8
NPC = N // NCORES
BLK, HB = 4, 8
R = 14  # padded window rows per blk
ROWS = 16  # lead pad + 14 + spare
IN_PITCH = NPC * G * ROWS * W  # 8192 per-partition elems of in_nc
W_PITCH = NPC * K * K * HB * W  # 25088

# (kw, n) fats offloaded to gpsimd; consumed by the PE late (not last).
# Empty: gpsimd tensor_tensor contends with DVE for the shared SBUF port
# (measured 1.3-1.5x DVE fat slowdown) -- a net loss.
_GPS_FATS = ()
_PROD_BUFS = 3
_SPLIT_TAIL = True  # split first/last kw fats per image into kh halves

_cache = {}


def _build_v3():
    import concourse.bacc as bacc
    import concourse.mybir as mybir
    import concourse.tile as tile
    from concourse.bass import AP

    fp32 = mybir.dt.float32
    fp16 = mybir.dt.float16
    mult = mybir.AluOpType.mult

    nc = bacc.Bacc("TRN2", target_bir_lowering=False, debug=False, num_devices=NCORES)
    x = nc.dram_tensor("input", [NPC, 128, G * ROWS * W], fp16, kind="ExternalInput").ap()
    wt = nc.dram_tensor("wt", [NPC, 128, K, K * HB * W], fp16, kind="ExternalInput").ap()
    idn = nc.dram_tensor("identity", [128, 128], fp16, kind="ExternalInput").ap()
    y = nc.dram_tensor("output", [NPC, C, H, W], fp32, kind="ExternalOutput").ap()

    GPS = list(_GPS_FATS)
    DVE_ORDER = [
        (3, 0), (3, 1), (2, 0), (4, 0), (2, 1), (4, 1),
        (1, 0), (5, 0), (1, 1), (5, 1),
    ] + [kn for kn in [(0, 0), (0, 1)] if kn not in GPS] + [(6, 0), (6, 1)]
    # PE consumption order: DVE order with gpsimd prods inserted late but not
    # last (they are ready mid-stream)
    PE_ORDER = [kn for kn in DVE_ORDER] + []
    if GPS:
        PE_ORDER = PE_ORDER[:-2] + GPS + PE_ORDER[-2:]

    with tile.TileContext(nc) as tc:
        with (
            tc.tile_pool(name="main", bufs=1) as pool,
            tc.tile_pool(name="prod", bufs=_PROD_BUFS) as ppool,
            tc.tile_pool(name="psum", bufs=1, space="PSUM") as pspool,
        ):
            in_nc = pool.tile([128, NPC, G, ROWS, W], fp16)
            w16 = pool.tile([128, NPC, K, K, HB, W], fp16)
            acc = pool.tile([128, NPC * G * HB * W], fp32)
            ident = pool.tile([128, 128], fp16)
            acc_ps = pspool.tile([128, NPC * G * HB * W], fp32)

            def _w_dma(eng, kw, n):
                dst = w16[:, n, kw].rearrange("p a h w -> p (a h w)")
                eng.dma_start(out=dst, in_=wt[n, :, kw])

            # All loads via HWDGE. Critical-path prefix in strict priority
            # order on the sync ring: FIFO per ring means in0 gets the full
            # SDMA bandwidth, and each DMA's completion receipt overlaps the
            # next one's transfer. (Splitting in0 across both rings, or
            # moving pieces to scalar/gpsimd, measured strictly worse.)
            # n=0 input in g-halves with the first chunk's kh-halves between
            # them: the first fat unit (kh 0..4, g 0..4) is gated only on
            # in0's first half + the chunk's first half -- minimal bytes.
            GRW4 = 4 * ROWS * W
            HW4 = 4 * HB * W
            nc.sync.dma_start(
                out=in_nc[:, 0, 0:4].rearrange("p g r w -> p (g r w)"),
                in_=x[0, :, 0:GRW4],
            )
            nc.sync.dma_start(
                out=w16[:, 0, 3, 0:4].rearrange("p a h w -> p (a h w)"),
                in_=wt[0, :, 3][:, 0:HW4],
            )
            nc.sync.dma_start(
                out=w16[:, 0, 3, 4:K].rearrange("p a h w -> p (a h w)"),
                in_=wt[0, :, 3][:, HW4:],
            )
            nc.sync.dma_start(
                out=in_nc[:, 0, 4:8].rearrange("p g r w -> p (g r w)"),
                in_=x[0, :, GRW4 : 2 * GRW4],
            )
            nc.sync.dma_start(out=ident[:], in_=idn[:])
            nc.sync.dma_start(
                out=in_nc[:, 1].rearrange("p g r w -> p (g r w)"), in_=x[1]
            )
            _w_dma(nc.sync, 3, 1)
            # The remaining chunks go on the scalar ring, gated behind the
            # ident DMA (tile schedules by dependency, so the gate must be a
            # real data dependency): warm copy reads ident, and a tiny guard
            # write into each chunk's first element reads warm, making every
            # chunk DMA (WAW on the guarded element) wait for ident to land.
            # This keeps the early SDMA bandwidth for the critical prefix.
            warm = pool.tile([128, 1], fp32)
            nc.scalar.copy(out=warm[:], in_=ident[:, 0:1])
            rest = [(2, 0), (4, 0), (2, 1), (4, 1), (1, 0), (5, 0),
                    (1, 1), (5, 1), (0, 0), (0, 1), (6, 0), (6, 1)]
            for kw, n in rest:
                guard = w16[:, n, kw].rearrange("p a h w -> p (a h w)")[:, 0:1]
                nc.scalar.mul(out=guard, in_=warm[:], mul=0.0)
                _w_dma(nc.scalar, kw, n)

            # DVE units: (kw, n, kh0, kh1, g0, g1). The first fats (pipeline
            # fill) and the last four fats are split into kh pieces: the PE
            # can only start consuming a unit once it fully completes, so
            # the end-of-stream PE drain is set by the size of the units on
            # its final critical path -- quarters for (6,*), halves for the
            # (0,*) fats just before them.
            def units_of(kw, n):
                if not _SPLIT_TAIL:
                    return [(kw, n, 0, K, 0, 8)]
                if _SPLIT_TAIL and (kw, n) == DVE_ORDER[-1]:
                    # g-pair-major: PSUM bank b (= g pair) finishes after its
                    # last kh piece, so its quarter can evict/store/receive
                    # while the DVE still computes the remaining g's -- only
                    # the final quarter's store chain stays exposed.
                    return [(kw, n, kh0, kh1, 2 * b, 2 * b + 2)
                            for b in range(4)
                            for kh0, kh1 in ((0, 4), (4, 6), (6, K))]
                if _SPLIT_TAIL and (kw, n) == DVE_ORDER[0]:
                    return [(kw, n, 0, 4, 0, 4), (kw, n, 0, 4, 4, 8),
                            (kw, n, 4, K, 0, 8)]
                if kw == DVE_ORDER[-1][0]:
                    return [(kw, n, 0, 2, 0, 8), (kw, n, 2, 4, 0, 8),
                            (kw, n, 4, 6, 0, 8), (kw, n, 6, K, 0, 8)]
                if kw in (DVE_ORDER[0][0], 0):
                    return [(kw, n, 0, 4, 0, 8), (kw, n, 4, K, 0, 8)]
                return [(kw, n, 0, K, 0, 8)]

            DVE_UNITS = [u for kn in DVE_ORDER for u in units_of(*kn)]
            # PE passes: one per DVE unit (keyed by the full 6-tuple); a
            # pass touches only the PSUM banks of its g-range. start/stop
            # flags are tracked per (n, bank): start on the bank's first
            # write, stop on its last (the final pass covering it).
            PE_UNITS = []
            pe_cover = {}
            for kn in PE_ORDER:
                for u in units_of(*kn):
                    key = u
                    if key not in pe_cover:
                        pe_cover[key] = 0
                        PE_UNITS.append(key)
                    pe_cover[key] += 1
            last_pass = {}  # (n, b) -> index in PE_UNITS of its final pass
            for idx, (kw, n, kh0, kh1, g0, g1) in enumerate(PE_UNITS):
                for b in range(g0 // 2, g1 // 2):
                    last_pass[(n, b)] = idx

            def fat_aps(kw, n, kh0, kh1, g0, g1, pb):
                v = in_nc[:]
                nk = kh1 - kh0
                in0 = AP(
                    v.tensor,
                    v.offset
                    + n * G * ROWS * W
                    + g0 * ROWS * W
                    + (ROWS - R - 1) * W
                    - PAD
                    + kw
                    + kh0 * W,
                    [[IN_PITCH, 128], [W, nk], [ROWS * W, g1 - g0], [1, HB * W]],
                )
                wv = w16[:]
                in1 = AP(
                    wv.tensor,
                    wv.offset
                    + n * K * K * HB * W
                    + kw * K * HB * W
                    + kh0 * HB * W,
                    [[W_PITCH, 128], [HB * W, nk], [0, g1 - g0], [1, HB * W]],
                )
                po = pb[:]
                outp = AP(
                    po.tensor,
                    po.offset + kh0 * G * HB * W + g0 * HB * W,
                    [[K * G * HB * W, 128], [G * HB * W, nk],
                     [HB * W, g1 - g0], [1, HB * W]],
                )
                return in0, in1, outp

            prods = {}
            unit_done = set()

            def emit_unit(kw, n, kh0, kh1, g0, g1, eng):
                if (kw, n) not in prods:
                    if (kw, n) in GPS:
                        pb = pool.tile(
                            [128, K, G * HB * W], fp16, tag=f"gps{kw}_{n}"
                        )
                    else:
                        pb = ppool.tile([128, K, G * HB * W], fp16)
                    prods[(kw, n)] = pb
                pb = prods[(kw, n)]
                in0, in1, outp = fat_aps(kw, n, kh0, kh1, g0, g1, pb)
                eng.tensor_tensor(out=outp, in0=in0, in1=in1, op=mult)
                key = (kw, n, kh0, kh1, g0, g1)
                pe_cover[key] -= 1
                if pe_cover[key] == 0:
                    unit_done.add(key)

            # gpsimd fats first in its program order
            for kw, n in GPS:
                for u in units_of(kw, n):
                    emit_unit(*u, nc.gpsimd)



            seen_banks = set()  # (n, b) whose PSUM bank has been started

            def pe_pass(pidx, kw, n, kh0, kh1, g0, g1):
                pb = prods[(kw, n)]
                pf = pb[:].rearrange("p k f -> p (k f)")
                for kh in range(kh0, kh1):
                    for b in range(g0 // 2, g1 // 2):
                        nc.tensor.matmul(
                            out=acc_ps[:, n * 2048 + b * 512 : n * 2048 + (b + 1) * 512],
                            lhsT=ident[:],
                            rhs=pf[:, kh * 2048 + b * 512 : kh * 2048 + (b + 1) * 512],
                            start=((n, b) not in seen_banks and kh == kh0),
                            stop=(last_pass[(n, b)] == pidx and kh == kh1 - 1),
                        )
                for b in range(g0 // 2, g1 // 2):
                    seen_banks.add((n, b))

            emitted = 0
            for kw, n, kh0, kh1, g0, g1 in DVE_UNITS:
                emit_unit(kw, n, kh0, kh1, g0, g1, nc.vector)
                # interleave PE passes as their units complete in program order
                while emitted < len(PE_UNITS):
                    u = PE_UNITS[emitted]
                    if u not in unit_done:
                        break
                    pe_pass(emitted, *u)
                    emitted += 1
            assert emitted == len(PE_UNITS), (emitted, len(PE_UNITS))

            # evict PSUM -> SBUF in per-n quarters alternating DVE/ACT; one
            # store DMA per quarter (spanning its 2 g's). Both evictions on
            # an engine come before its store gens so the last eviction is
            # not delayed behind descriptor generation.
            # DVE evicts q0,q2; ACT evicts q3 BEFORE q1 (bank q3 is written
            # last by the PE, so evicting it first lets its store gen issue
            # ~1us earlier and pulls in the final HBM write receipt)
            for n in range(NPC):
                dsty = y[n].rearrange(
                    "(g cc) (blk hb) w -> cc blk g (hb w)", g=G, blk=BLK
                )
                for q in (0, 2):
                    lo = n * 2048 + q * 512
                    nc.vector.tensor_copy(
                        out=acc[:, lo : lo + 512], in_=acc_ps[:, lo : lo + 512]
                    )
                for q in (1, 3):
                    lo = n * 2048 + q * 512
                    nc.scalar.copy(
                        out=acc[:, lo : lo + 512], in_=acc_ps[:, lo : lo + 512]
                    )
                for q in (0, 1, 2, 3):
                    lo = n * 2048 + q * 512
                    deng = nc.sync if q % 2 == 0 else nc.scalar
                    deng.dma_start(
                        out=dsty[:, :, 2 * q : 2 * q + 2],
                        in_=acc[:, lo : lo + 512].rearrange(
                            "p (g f) -> p g f", g=2
                        ),
                    )

    nc.compile()
    return nc


def _get_nc():
    if "nc" not in _cache:
        _cache["nc"] = _build_v3()
    return _cache["nc"]


def _prep_weight(weight):
    # [N, CC, KK, H, W] -> [N, (cc blk), kw, kh, hb, w] fp16, edge out-columns
    # of each kw zeroed (exact: they multiply conv-padding zeros)
    w = weight.reshape(N, CC, K, K, BLK, HB, W)  # [n, cc, kh, kw, blk, hb, w]
    w = np.ascontiguousarray(w.transpose(0, 1, 4, 3, 2, 5, 6))
    # -> [n, cc, blk, kw, kh, hb, w]
    for kw in range(K):
        if kw < PAD:
            w[:, :, :, kw, :, :, 0 : PAD - kw] = 0.0
        elif kw > PAD:
            w[:, :, :, kw, :, :, W + PAD - kw : W] = 0.0
    return w.reshape(N, 128, K, K * HB * W).astype(np.float16)


def _prep_input(input_):
    # [N, C, H, W] f32 -> [N, (cc blk), (g rows w)] fp16 with halo rows
    # pre-zeroed, matching the in_nc SBUF layout exactly.
    xt = input_.reshape(N, G, CC, H, W).transpose(0, 2, 1, 3, 4)  # [n,cc,g,h,w]
    arr = np.zeros((N, CC, BLK, G, ROWS, W), dtype=np.float16)
    for blk in range(BLK):
        h0 = max(0, blk * HB - PAD)
        h1 = min(H, blk * HB - PAD + R)
        r0 = h0 - (blk * HB - PAD)
        arr[:, :, blk, :, 1 + r0 : 1 + r0 + (h1 - h0), :] = xt[:, :, :, h0:h1, :]
    return arr.reshape(N, 128, G * ROWS * W)


def kernel(input_, weight, _trace=False):
    from concourse.bass_utils import run_bass_kernel_spmd

    nc = _get_nc()
    input_ = np.ascontiguousarray(input_, dtype=np.float32)
    weight = np.ascontiguousarray(weight, dtype=np.float32)
    xh = _prep_input(input_)
    wh = _prep_weight(weight)
    eye = np.eye(128, dtype=np.float16)
    in_maps = [
        {
            "input": xh[i * NPC : (i + 1) * NPC],
            "wt": wh[i * NPC : (i + 1) * NPC],
            "identity": eye,
        }
        for i in range(NCORES)
    ]
    res = run_bass_kernel_spmd(nc, in_maps, list(range(NCORES)), trace=_trace)
    _cache["last_result"] = res
    out = np.concatenate([res.results[i]["output"] for i in range(NCORES)], axis=0)
    return out


# revision 44
# speedup vs baseline: 1.0281x; 1.0281x over previous
"""Trainium2 Bass kernel for nn_Aggregation (SAN-style local aggregation).

out[n, g*32+cc, h, w] = sum_{kh,kw} input[n, g*32+cc, h-3+kh, w-3+kw] * weight[n, cc, kh*7+kw, h, w]

Sharding: data-parallel over batch N=16 across 8 NeuronCores (2 images/core).

v3 layout (per core):
  partition p = cc*4 + blk   (cc in [0,32): weight channel, blk in [0,4): block of 8 output rows)
  in_nc[p][n, g, rho, w]: rho = 1 + r, r in [0,14) the padded window rows
    (lead row rho=0 and spare row rho=15 absorb column spill), w unpadded.
  Input is HOST-prepared: fp16, halo rows pre-zeroed, exact SBUF layout ->
  one plain 2D HWDGE DMA per image (no cast, no memset, no SWDGE).
  Weight is host-permuted to [n, (cc blk), kw, kh, hb, w] fp16 with the edge
  output-columns of each kw-tap zeroed host-side (exact: those weights
  multiply conv padding zeros in the reference).
  Engines: DVE computes all 14 fat tensor_tensor multiplies (free dims
  kh:7, g:8, (hb w):256 -- 2x_1P mode, fully dense back-to-back). All DMAs
  are HWDGE (sync/scalar rings) so nothing contends with the DVE; the 12
  non-critical weight chunks are dependency-gated behind the ident load so
  the n=0 input + first chunk get the full SDMA bandwidth at startup.
  Products are fp16; the Tensor engine accumulates all taps into an fp32
  PSUM accumulator via identity matmuls, chased by DVE/ACT PSUM evictions
  and per-quarter output stores.

  Measured (8 cores, this config): ~128.9-129.6us HW exec (cold device;
  +-1us run-to-run DMA-receipt jitter). Breakdown: ~7.2us fixed engine
  preamble, ~6.1us first-data DMA latency (latency-bound: halving the
  gating bytes moves the gate <0.1us), 106.4us DVE multiplies (the
  2 elem/cycle/partition 2x_1P roofline -- 25.7M products/core), ~1.1us
  PE drain (single-kh final units), ~8us evict/store/receipt/cleanup. The chip thermally
  throttles ~0.83x (both PE and DVE; ~153us) after many back-to-back
  runs; ~5min idle restores full clocks. Things measured WORSE: gpsimd
  tensor_tensor offload (shared-SBUF-port contention slows DVE ~0.6x per
  gpsimd-busy-us), SWDGE/gpsimd DMA issue (late descriptor gen + big
  drains), splitting the n=0 input DMA across both HWDGE rings.
"""

import numpy as np

N, C, H, W = 16, 256, 32, 32
K, PAD = 7, 3
CC, G = 32, 8
KK = K * K
NCORES = 8
NPC = N // NCORES
BLK, HB = 4, 8
R = 14  # padded window rows per blk
ROWS = 16  # lead pad + 14 + spare
IN_PITCH = NPC * G * ROWS * W  # 8192 per-partition elems of in_nc
W_PITCH = NPC * K * K * HB * W  # 25088

# (kw, n) fats offloaded to gpsimd; consumed by the PE late (not last).
# Empty: gpsimd tensor_tensor contends with DVE for the shared SBUF port
# (measured 1.3-1.5x DVE fat slowdown) -- a net loss.
_GPS_FATS = ()
_PROD_BUFS = 3
_SPLIT_TAIL = True  # split first/last kw fats per image into kh halves

_cache = {}


def _build_v3():
    import concourse.bacc as bacc
    import concourse.mybir as mybir
    import concourse.tile as tile
    from concourse.bass import AP

    fp32 = mybir.dt.float32
    fp16 = mybir.dt.float16
    mult = mybir.AluOpType.mult

    nc = bacc.Bacc("TRN2", target_bir_lowering=False, debug=False, num_devices=NCORES)
    x = nc.dram_tensor("input", [NPC, 128, G * ROWS * W], fp16, kind="ExternalInput").ap()
    wt = nc.dram_tensor("wt", [NPC, 128, K, K * HB * W], fp16, kind="ExternalInput").ap()
    idn = nc.dram_tensor("identity", [128, 128], fp16, kind="ExternalInput").ap()
    y = nc.dram_tensor("output", [NPC, C, H, W], fp32, kind="ExternalOutput").ap()

    GPS = list(_GPS_FATS)
    DVE_ORDER = [
        (3, 0), (3, 1), (2, 0), (4, 0), (2, 1), (4, 1),
        (1, 0), (5, 0), (1, 1), (5, 1),
    ] + [kn for kn in [(0, 0), (0, 1)] if kn not in GPS] + [(6, 0), (6, 1)]
    # PE consumption order: DVE order with gpsimd prods inserted late but not
    # last (they are ready mid-stream)
    PE_ORDER = [kn for kn in DVE_ORDER] + []
    if GPS:
        PE_ORDER = PE_ORDER[:-2] + GPS + PE_ORDER[-2:]

    with tile.TileContext(nc) as tc:
        with (
            tc.tile_pool(name="main", bufs=1) as pool,
            tc.tile_pool(name="prod", bufs=_PROD_BUFS) as ppool,
            tc.tile_pool(name="psum", bufs=1, space="PSUM") as pspool,
        ):
            in_nc = pool.tile([128, NPC, G, ROWS, W], fp16)
            w16 = pool.tile([128, NPC, K, K, HB, W], fp16)
            acc = pool.tile([128, NPC * G * HB * W], fp32)
            ident = pool.tile([128, 128], fp16)
            acc_ps = pspool.tile([128, NPC * G * HB * W], fp32)

            def _w_dma(eng, kw, n):
                dst = w16[:, n, kw].rearrange("p a h w -> p (a h w)")
                eng.dma_start(out=dst, in_=wt[n, :, kw])

            # All loads via HWDGE. Critical-path prefix in strict priority
            # order on the sync ring: FIFO per ring means in0 gets the full
            # SDMA bandwidth, and each DMA's completion receipt overlaps the
            # next one's transfer. (Splitting in0 across both rings, or
            # moving pieces to scalar/gpsimd, measured strictly worse.)
            # n=0 input in g-halves with the first chunk's kh-halves between
            # them: the first fat unit (kh 0..4, g 0..4) is gated only on
            # in0's first half + the chunk's first half -- minimal bytes.
            GRW4 = 4 * ROWS * W
            HW4 = 4 * HB * W
            nc.sync.dma_start(
                out=in_nc[:, 0, 0:4].rearrange("p g r w -> p (g r w)"),
                in_=x[0, :, 0:GRW4],
            )
            nc.sync.dma_start(
                out=w16[:, 0, 3, 0:4].rearrange("p a h w -> p (a h w)"),
                in_=wt[0, :, 3][:, 0:HW4],
            )
            nc.sync.dma_start(
                out=w16[:, 0, 3, 4:K].rearrange("p a h w -> p (a h w)"),
                in_=wt[0, :, 3][:, HW4:],
            )
            nc.sync.dma_start(
                out=in_nc[:, 0, 4:8].rearrange("p g r w -> p (g r w)"),
                in_=x[0, :, GRW4 : 2 * GRW4],
            )
            nc.sync.dma_start(out=ident[:], in_=idn[:])
            nc.sync.dma_start(
                out=in_nc[:, 1].rearrange("p g r w -> p (g r w)"), in_=x[1]
            )
            _w_dma(nc.sync, 3, 1)
            # The remaining chunks go on the scalar ring, gated behind the
            # ident DMA (tile schedules by dependency, so the gate must be a
            # real data dependency): warm copy reads ident, and a tiny guard
            # write into each chunk's first element reads warm, making every
            # chunk DMA (WAW on the guarded element) wait for ident to land.
            # This keeps the early SDMA bandwidth for the critical prefix.
            warm = pool.tile([128, 1], fp32)
            nc.scalar.copy(out=warm[:], in_=ident[:, 0:1])
            rest = [(2, 0), (4, 0), (2, 1), (4, 1), (1, 0), (5, 0),
                    (1, 1), (5, 1), (0, 0), (0, 1), (6, 0), (6, 1)]
            for kw, n in rest:
                guard = w16[:, n, kw].rearrange("p a h w -> p (a h w)")[:, 0:1]
                nc.scalar.mul(out=guard, in_=warm[:], mul=0.0)
                _w_dma(nc.scalar, kw, n)

            # DVE units: (kw, n, kh0, kh1, g0, g1). The first fats (pipeline
            # fill) and the last four fats are split into kh pieces: the PE
            # can only start consuming a unit once it fully completes, so
            # the end-of-stream PE drain is set by the size of the units on
            # its final critical path -- quarters for (6,*), halves for the
            # (0,*) fats just before them.
            def units_of(kw, n):
                if not _SPLIT_TAIL:
                    return [(kw, n, 0, K, 0, 8)]
                if _SPLIT_TAIL and (kw, n) == DVE_ORDER[-1]:
                    # g-pair-major: PSUM bank b (= g pair) finishes after its
                    # last kh piece, so its quarter can evict/store/receive
                    # while the DVE still computes the remaining g's -- only
                    # the final quarter's store chain stays exposed.
                    return [(kw, n, kh0, kh1, 2 * b, 2 * b + 2)
                            for b in range(4)
                            for kh0, kh1 in ((0, 4), (4, 6), (6, K))]
                if _SPLIT_TAIL and (kw, n) == DVE_ORDER[0]:
                    return [(kw, n, 0, 4, 0, 4), (kw, n, 0, 4, 4, 8),
                            (kw, n, 4, K, 0, 8)]
                if kw == DVE_ORDER[-1][0]:
                    return [(kw, n, 0, 2, 0, 8), (kw, n, 2, 4, 0, 8),
                            (kw, n, 4, 6, 0, 8), (kw, n, 6, K, 0, 8)]
                if kw in (DVE_ORDER[0][0], 0):
                    return [(kw, n, 0, 4, 0, 8), (kw, n, 4, K, 0, 8)]
                return [(kw, n, 0, K, 0, 8)]

            DVE_UNITS = [u for kn in DVE_ORDER for u in units_of(*kn)]
            # PE passes: one per DVE unit (keyed by the full 6-tuple); a
            # pass touches only the PSUM banks of its g-range. start/stop
            # flags are tracked per (n, bank): start on the bank's first
            # write, stop on its last (the final pass covering it).
            PE_UNITS = []
            pe_cover = {}
            for kn in PE_ORDER:
                for u in units_of(*kn):
                    key = u
                    if key not in pe_cover:
                        pe_cover[key] = 0
                        PE_UNITS.append(key)
                    pe_cover[key] += 1
            last_pass = {}  # (n, b) -> index in PE_UNITS of its final pass
            for idx, (kw, n, kh0, kh1, g0, g1) in enumerate(PE_UNITS):
                for b in range(g0 // 2, g1 // 2):
                    last_pass[(n, b)] = idx

            def fat_aps(kw, n, kh0, kh1, g0, g1, pb):
                v = in_nc[:]
                nk = kh1 - kh0
                in0 = AP(
                    v.tensor,
                    v.offset
                    + n * G * ROWS * W
                    + g0 * ROWS * W
                    + (ROWS - R - 1) * W
                    - PAD
                    + kw
                    + kh0 * W,
                    [[IN_PITCH, 128], [W, nk], [ROWS * W, g1 - g0], [1, HB * W]],
                )
                wv = w16[:]
                in1 = AP(
                    wv.tensor,
                    wv.offset
                    + n * K * K * HB * W
                    + kw * K * HB * W
                    + kh0 * HB * W,
                    [[W_PITCH, 128], [HB * W, nk], [0, g1 - g0], [1, HB * W]],
                )
                po = pb[:]
                outp = AP(
                    po.tensor,
                    po.offset + kh0 * G * HB * W + g0 * HB * W,
                    [[K * G * HB * W, 128], [G * HB * W, nk],
                     [HB * W, g1 - g0], [1, HB * W]],
                )
                return in0, in1, outp

            prods = {}
            unit_done = set()

            def emit_unit(kw, n, kh0, kh1, g0, g1, eng):
                if (kw, n) not in prods:
                    if (kw, n) in GPS:
                        pb = pool.tile(
                            [128, K, G * HB * W], fp16, tag=f"gps{kw}_{n}"
                        )
                    else:
                        pb = ppool.tile([128, K, G * HB * W], fp16)
                    prods[(kw, n)] = pb
                pb = prods[(kw, n)]
                in0, in1, outp = fat_aps(kw, n, kh0, kh1, g0, g1, pb)
                eng.tensor_tensor(out=outp, in0=in0, in1=in1, op=mult)
                key = (kw, n, kh0, kh1, g0, g1)
                pe_cover[key] -= 1
                if pe_cover[key] == 0:
                    unit_done.add(key)

            # gpsimd fats first in its program order
            for kw, n in GPS:
                for u in units_of(kw, n):
                    emit_unit(*u, nc.gpsimd)



            seen_banks = set()  # (n, b) whose PSUM bank has been started

            def pe_pass(pidx, kw, n, kh0, kh1, g0, g1):
                pb = prods[(kw, n)]
                pf = pb[:].rearrange("p k f -> p (k f)")
                for kh in range(kh0, kh1):
                    for b in range(g0 // 2, g1 // 2):
                        nc.tensor.matmul(
                            out=acc_ps[:, n * 2048 + b * 512 : n * 2048 + (b + 1) * 512],
                            lhsT=ident[:],
                            rhs=pf[:, kh * 2048 + b * 512 : kh * 2048 + (b + 1) * 512],
                            start=((n, b) not in seen_banks and kh == kh0),
                            stop=(last_pass[(n, b)] == pidx and kh == kh1 - 1),
                        )
                for b in range(g0 // 2, g1 // 2):
                    seen_banks.add((n, b))

            emitted = 0
            for kw, n, kh0, kh1, g0, g1 in DVE_UNITS:
                emit_unit(kw, n, kh0, kh1, g0, g1, nc.vector)
                # interleave PE passes as their units complete in program order
                while emitted < len(PE_UNITS):
                    u = PE_UNITS[emitted]
                    if u not in unit_done:
                        break
                    pe_pass(emitted, *u)
                    emitted += 1
            assert emitted == len(PE_UNITS), (emitted, len(PE_UNITS))

            # evict PSUM -> SBUF in per-n quarters alternating DVE/ACT; one
            # store DMA per quarter (spanning its 2 g's). Both evictions on
            # an engine come before its store gens so the last eviction is
            # not delayed behind descriptor generation.
            # ALL evictions on ACT (the DVE's serial queue is busy with fats
            # until the very end, so DVE-assigned evicts would queue behind
            # them; ACT is free and each bank's evict fires as soon as its
            # g-pair completes). ALL store gens on the idle sync ring.
            for n in range(NPC):
                dsty = y[n].rearrange(
                    "(g cc) (blk hb) w -> cc blk g (hb w)", g=G, blk=BLK
                )
                for q in range(4):
                    lo = n * 2048 + q * 512
                    nc.scalar.copy(
                        out=acc[:, lo : lo + 512], in_=acc_ps[:, lo : lo + 512]
                    )
                    nc.sync.dma_start(
                        out=dsty[:, :, 2 * q : 2 * q + 2],
                        in_=acc[:, lo : lo + 512].rearrange(
                            "p (g f) -> p g f", g=2
                        ),
                    )

    nc.compile()
    return nc


def _get_nc():
    if "nc" not in _cache:
        _cache["nc"] = _build_v3()
    return _cache["nc"]


def _prep_weight(weight):
    # [N, CC, KK, H, W] -> [N, (cc blk), kw, kh, hb, w] fp16, edge out-columns
    # of each kw zeroed (exact: they multiply conv-padding zeros)
    w = weight.reshape(N, CC, K, K, BLK, HB, W)  # [n, cc, kh, kw, blk, hb, w]
    w = np.ascontiguousarray(w.transpose(0, 1, 4, 3, 2, 5, 6))
    # -> [n, cc, blk, kw, kh, hb, w]
    for kw in range(K):
        if kw < PAD:
            w[:, :, :, kw, :, :, 0 : PAD - kw] = 0.0
        elif kw > PAD:
            w[:, :, :, kw, :, :, W + PAD - kw : W] = 0.0
    return w.reshape(N, 128, K, K * HB * W).astype(np.float16)


def _prep_input(input_):
    # [N, C, H, W] f32 -> [N, (cc blk), (g rows w)] fp16 with halo rows
    # pre-zeroed, matching the in_nc SBUF layout exactly.
    xt = input_.reshape(N, G, CC, H, W).transpose(0, 2, 1, 3, 4)  # [n,cc,g,h,w]
    arr = np.zeros((N, CC, BLK, G, ROWS, W), dtype=np.float16)
    for blk in range(BLK):
        h0 = max(0, blk * HB - PAD)
        h1 = min(H, blk * HB - PAD + R)
        r0 = h0 - (blk * HB - PAD)
        arr[:, :, blk, :, 1 + r0 : 1 + r0 + (h1 - h0), :] = xt[:, :, :, h0:h1, :]
    return arr.reshape(N, 128, G * ROWS * W)


def kernel(input_, weight, _trace=False):
    from concourse.bass_utils import run_bass_kernel_spmd

    nc = _get_nc()
    input_ = np.ascontiguousarray(input_, dtype=np.float32)
    weight = np.ascontiguousarray(weight, dtype=np.float32)
    xh = _prep_input(input_)
    wh = _prep_weight(weight)
    eye = np.eye(128, dtype=np.float16)
    in_maps = [
        {
            "input": xh[i * NPC : (i + 1) * NPC],
            "wt": wh[i * NPC : (i + 1) * NPC],
            "identity": eye,
        }
        for i in range(NCORES)
    ]
    res = run_bass_kernel_spmd(nc, in_maps, list(range(NCORES)), trace=_trace)
    _cache["last_result"] = res
    out = np.concatenate([res.results[i]["output"] for i in range(NCORES)], axis=0)
    return out
